# revision 76
# baseline (speedup 1.0000x reference)
"""Multi-head attention with additive positional attention — TRN2 Bass kernel.

Problem: B=4, S=2048, DM=128, H=8, DK=16.
  scores = (q @ k^T) / sqrt(DK) + pos_q @ pos_k^T   per (b, h)
  out    = softmax(scores) @ v, heads merged, @ Wo^T + bo

Sharding: 8 cores = batch (4) x query-row halves (2). Each core holds one
batch's full keys/values (S=2048) and 1024 query rows, computes all 8 heads,
and produces complete output rows — no cross-core reduction; the host gather
is a pure concatenation.

Per-core algorithm (all feature-major "T" layouts = [feature, seq]):
  - kcat/qcat: per head h, a 32-partition block [k_h (16 rows); pos_k_h (16)]
    (resp. [q_h * scale; pos_q_h]) so scoresT = kcat_blk^T @ qcat_blk fuses
    the qk and positional terms into ONE K=32 matmul per tile, 4 heads
    row-tiled concurrently on the PE's 32-row groups.
  - softmax without max-subtraction (scores are O(30), exp is fp32-safe).
  - v is augmented to 32 columns per head [1 | v_h | 0*15]: attn@v, the
    softmax row-sums, and hard zeros for the padding rows all come from one
    col-tiled matmul (M=32).
  - normalization (divide by row-sum) commutes with the output projection,
    so it's applied once at the end; Wo is host-permuted to read the
    scattered [head-block @ 32j] layout directly.
"""

import numpy as np

H, DK, DM = 8, 16, 128
B, S = 4, 2048
R = 1024  # query rows per core
NCORES = 8
NKC = S // 128  # 16 key chunks
NQC = R // 512  # 2 q chunks

_CACHE = {}


def _patch_drain():
    """walrus on this stack rejects >1 sync-wait on CTRL instructions; the
    TileContext exit drain can carry several. Absorb them on SP nops first."""
    import concourse.mybir as mybir
    from concourse.tile import TileContext, ScopedClock

    if getattr(TileContext, "_drain_patched", False):
        return
    orig = TileContext._drain_and_barrier

    def patched(self, tick_clock, wait_clock):
        nc = self.nc
        probe = nc.sync.nop(nofuse=True)
        wait_clock.add_sem_waits(
            probe.ins, ScopedClock({None: tick_clock.global_clock})
        )
        w = list(probe.ins.sync_info.on_wait or []) if probe.ins.sync_info else []
        if len(w) > 1:
            probe.ins.sync_info.on_wait = w[:1]
            for i in range(1, len(w)):
                n2 = nc.sync.nop(nofuse=True)
                n2.ins.sync_info = mybir.SyncInfo(on_wait=w[i : i + 1], on_update=[])

        class _NoWaits:
            def __init__(s, real):
                s._real = real

            def add_sem_waits(s, ins, clock):
                pass

            def __getattr__(s, k):
                return getattr(s._real, k)

        orig(self, tick_clock, _NoWaits(wait_clock))

    TileContext._drain_and_barrier = patched
    TileContext._drain_patched = True


def _split_multi_waits(nc, mybir):
    """walrus here accepts at most 1 sync-wait on most instruction structs
    (2 on EventSemaphore). Hoist excess waits onto same-engine NoOps placed
    immediately before the instruction — same blocking semantics."""
    for f in nc.m.functions:
        for blk in f.blocks:
            new_insts = []
            changed = False
            for inst in blk.instructions:
                si = inst.sync_info
                waits = list(si.on_wait) if si and si.on_wait else []
                limit = 2 if type(inst).__name__ == "InstEventSemaphore" else 1
                if len(waits) > limit:
                    changed = True
                    extra = waits[: len(waits) - limit]
                    for wv in extra:
                        n = mybir.InstNoOp(
                            name=f"wsplit_{nc.next_id()}",
                            engine=inst.engine,
                            ins=[],
                            outs=[],
                            sync_info=mybir.SyncInfo(on_wait=[wv], on_update=[]),
                        )
                        nc.register_instruction(n)
                        new_insts.append(n)
                    inst.sync_info.on_wait = waits[len(waits) - limit :]
                new_insts.append(inst)
            if changed:
                blk.instructions = new_insts


def build_bass(mm_dtype="float32"):
    import concourse.bass as bass
    import concourse.mybir as mybir
    import concourse.tile as tile

    _patch_drain()
    dt = mybir.dt
    f32 = dt.float32
    # mdt: dtype of every tensor that feeds a matmul. float32r = fp32 with
    # 11-bit mantissa (PE fast path, 1 cyc/col vs 4 for fp32 at N>=256).
    # The BIR verifier requires each matmul operand's PRODUCER to declare
    # float32r output (engines round on write; DMA chains end at an f32r
    # ExternalInput that the host pre-rounds).
    mdt = getattr(dt, mm_dtype)
    # f32r matmuls require dst partition offset 0 — the 4 col-tiled av
    # matmuls write bands at partitions 32j, so they use bf16 operands
    # instead (same 1 cyc/col rate, no dst restriction; attn weights and v
    # tolerate bf16 rounding easily at rel_fro < 2e-2).
    avdt = dt.bfloat16 if mdt != f32 else f32
    AF = mybir.ActivationFunctionType
    OP = mybir.AluOpType

    def fv(ap):  # f32 view of a (possibly f32r) AP for non-matmul readers
        return ap if mdt == f32 else ap.bitcast(f32)

    nc = bass.Bass("TRN2", num_devices=NCORES, enable_asserts=False)

    def inp(name, shape, dtype=f32):
        return nc.dram_tensor(name, shape, dtype, kind="ExternalInput")

    xqT_d = inp("xqT", [DM, R], mdt)
    posqT_d = inp("posqT", [DM, R], mdt)
    xkT_d = inp("xkT", [DM, S], mdt)
    xvT_d = inp("xvT", [DM, S], mdt)
    posT_d = inp("posT", [DM, S], mdt)
    w0Ts_d = inp("w0Ts", [DM, DM], mdt)
    w0T_d = inp("w0T", [DM, DM], mdt)
    w1T_d = inp("w1T", [DM, DM], mdt)
    w2T_d = inp("w2T", [DM, DM], mdt)
    woPA_d = inp("woPA", [DM, DM], mdt)
    woPB_d = inp("woPB", [DM, DM], mdt)
    # biases packed into one [DM, 4] DMA (cols: b0s, b0c, b1c, bo) and b2 as
    # a single-partition row (replicated on-device) — [DM,1] loads cost 128
    # tiny descriptors each and serialized ~2.5us apiece on the SP ring
    bpack_d = inp("bpack", [DM, 4])
    b2row_d = inp("b2row", [1, DM])
    outT_d = nc.dram_tensor("outT", [DM, R], f32, kind="ExternalOutput")
    sums_d = nc.dram_tensor("sums_scratch", [H, R], mdt)

    with tile.TileContext(nc) as tc:
        with (
            tc.tile_pool(name="singles", bufs=1) as singles,
            tc.tile_pool(name="exps", bufs=4) as exps,
            tc.tile_pool(name="tailp", bufs=1) as tailp,
        ):
            # ---------------- input loads ----------------
            # loads: SP ring (need-ordered); ACT ring carries only the
            # phase-1 interleave pieces (its in-order queue must never make
            # a load wait behind a piece that needs projection results)
            _ring = [nc.sync, nc.scalar]

            def load(dram, shape, ring=None):
                t = singles.tile(
                    shape, dram.dtype, tag=dram.name + "_s", name=dram.name + "_s"
                )
                (ring or nc.sync).dma_start(out=t[:, :], in_=dram[:, :])
                return t

            s_w1T = load(w1T_d, [DM, DM])
            s_xkT = load(xkT_d, [DM, S], nc.scalar)
            s_w0Ts = load(w0Ts_d, [DM, DM])
            s_xqT = load(xqT_d, [DM, R], nc.scalar)
            s_bpack = load(bpack_d, [DM, 4])
            s_posT = load(posT_d, [DM, S], nc.scalar)
            s_w0T = load(w0T_d, [DM, DM])
            s_posqT = load(posqT_d, [DM, R], nc.scalar)
            s_w2T = load(w2T_d, [DM, DM])
            s_xvT = load(xvT_d, [DM, S], nc.scalar)
            s_woPA = load(woPA_d, [DM, DM])
            s_woPB = load(woPB_d, [DM, DM], nc.scalar)
            s_b0s = s_bpack[:, 0:1]
            s_b0c = s_bpack[:, 1:2]
            s_b1c = s_bpack[:, 2:3]
            s_boc = s_bpack[:, 3:4]
            s_b2r = singles.tile([DM, DM], f32, tag="b2r_s", name="b2r_s")
            nc.scalar.dma_start(
                out=s_b2r[:, :], in_=b2row_d[0:1, :].broadcast_to((DM, DM))
            )

            kcat = [
                singles.tile([DM, S], mdt, tag="kcatA", name="kcatA"),
                singles.tile([DM, S], mdt, tag="kcatB", name="kcatB"),
            ]
            qcat = [
                singles.tile([DM, R], mdt, tag="qcatA", name="qcatA"),
                singles.tile([DM, R], mdt, tag="qcatB", name="qcatB"),
            ]
            # per head: [v_h (16) | 1 | 0*15] -> av matmul M=32 writes hard
            # zeros into the padding rows of each 32-block
            v_aug = singles.tile([DM, NKC, 32 * H], avdt, tag="vaug", name="vaug")
            xs = [
                singles.tile([DM, R], mdt, tag="xsA", name="xsA"),
                singles.tile([DM, R], mdt, tag="xsB", name="xsB"),
            ]
            nc.gpsimd.memset(v_aug[:, :, :], 0.0)
            nc.gpsimd.memset(
                v_aug.rearrange("p t (h u) -> p t h u", u=32)[:, :, :, 0], 1.0
            )

            # ---------------- projections ----------------
            # full feature-major projections into SBUF scratch, then DMA
            # partition-interleave into the per-head-block kcat/qcat layout
            kT_sb = singles.tile([DM, S], mdt, tag="kT_sb", name="kT_sb")
            pkT_sb = singles.tile([DM, S], mdt, tag="pkT_sb", name="pkT_sb")
            qT_sb = singles.tile([DM, R], mdt, tag="qT_sb", name="qT_sb")
            pqT_sb = singles.tile([DM, R], mdt, tag="pqT_sb", name="pqT_sb")

            with tc.tile_pool(name="proj_psum", bufs=4, space="PSUM") as proj_psum:
                def proj_chunk(lhsT, rhs_src, c0, bias, dst_sb):
                    pk = proj_psum.tile([128, 512], f32, tag="proj", name="pk")
                    nc.tensor.matmul(
                        out=pk[:, :],
                        lhsT=lhsT[:, :],
                        rhs=rhs_src[:, c0 : c0 + 512],
                        start=True,
                        stop=True,
                    )
                    # evac + bias in one op
                    nc.vector.tensor_scalar_add(
                        out=dst_sb[:, c0 : c0 + 512],
                        in0=pk[:, :],
                        scalar1=bias[:, :],
                    )

                # interleave pieces: kcat[g][32j + 16half + d] =
                # (k if half==0 else pos_k)[16(4g+j)+d], per (j, half),
                # column-phased so early-needed ranges land first
                def kpieces(g, cols, rings):
                    for j in range(4):
                        h = 4 * g + j
                        for half, src in ((0, kT_sb), (1, pkT_sb)):
                            r0 = 32 * j + 16 * half
                            rings[(j + half) % len(rings)].dma_start(
                                out=kcat[g][r0 : r0 + 16, cols],
                                in_=src[16 * h : 16 * h + 16, cols],
                            )

                def qpieces(g, cols, rings):
                    for j in range(4):
                        h = 4 * g + j
                        for half, src in ((0, qT_sb), (1, pqT_sb)):
                            r0 = 32 * j + 16 * half
                            rings[(j + half) % len(rings)].dma_start(
                                out=qcat[g][r0 : r0 + 16, cols],
                                in_=src[16 * h : 16 * h + 16, cols],
                            )

                # round-robin chunk 0 of each projection, then phase-1 pieces
                both = [nc.sync, nc.scalar]
                sp = [nc.sync]
                proj_chunk(s_w1T, s_xkT, 0, s_b1c, kT_sb)
                proj_chunk(s_w1T, s_posT, 0, s_b1c, pkT_sb)
                proj_chunk(s_w0Ts, s_xqT, 0, s_b0s, qT_sb)
                proj_chunk(s_w0T, s_posqT, 0, s_b0c, pqT_sb)
                kpieces(0, slice(0, 512), both)
                qpieces(0, slice(0, 512), both)
                proj_chunk(s_w1T, s_xkT, 512, s_b1c, kT_sb)
                proj_chunk(s_w1T, s_posT, 512, s_b1c, pkT_sb)
                proj_chunk(s_w0Ts, s_xqT, 512, s_b0s, qT_sb)
                proj_chunk(s_w0T, s_posqT, 512, s_b0c, pqT_sb)
                qpieces(0, slice(512, R), both)
                for c0 in (1024, 1536):
                    proj_chunk(s_w1T, s_xkT, c0, s_b1c, kT_sb)
                    proj_chunk(s_w1T, s_posT, c0, s_b1c, pkT_sb)
                kpieces(0, slice(512, S), sp)

                # v projection for the first two key chunks only — the rest
                # is pipelined into the attention loop (PE has slack there)
                def vproj(t, psum_tile, pcols):
                    nc.tensor.matmul(
                        out=psum_tile[:, pcols],
                        lhsT=s_xvT[:, t * 128 : (t + 1) * 128],
                        rhs=s_w2T[:, :],
                        start=True,
                        stop=True,
                    )
                    nc.vector.tensor_tensor(
                        out=v_aug.rearrange("p t (h u) -> p t h u", u=32)[
                            :, t, :, 1:17
                        ],
                        in0=psum_tile[:, pcols].rearrange(
                            "p (h u) -> p h u", u=16
                        ),
                        in1=s_b2r.rearrange("p (h u) -> p h u", u=16),
                        op=OP.add,
                    )

                for t in range(NKC):
                    pv = proj_psum.tile([128, 512], f32, tag="proj", name="pv")
                    vproj(t, pv, slice(0, DM))

                # phase-2 interleave for g=1: SP ring, hides under the loop
                kpieces(1, slice(0, S), sp)
                qpieces(1, slice(0, R), sp)

            # ---------------- attention main loop ----------------
            # qc-outer: each (g,qc) output quarter finalizes at 1/4-points of
            # the loop, so its normalize chain hides under later iterations
            ITERS = [
                (g, kc, qc) for g in (0, 1) for qc in range(NQC) for kc in range(NKC)
            ]
            T = len(ITERS)
            sct = {}

            _srs = {}

            def tail_norm_scatter(g, qc):
                # one batched DMA: the 4 head row-sums (rows 32j of xs) ->
                # DRAM bounce rows. SP ring only (ACT ring stalls mid-loop
                # exps).
                sl = slice(qc * 512, (qc + 1) * 512)
                for j in range(4):
                    h = 4 * g + j
                    nc.sync.dma_start(
                        out=sums_d[h : h + 1, sl],
                        in_=xs[g][32 * j : 32 * j + 1, sl],
                    )
                _srs[(g, qc)] = tailp.tile(
                    [DM, 512], mdt, tag=f"sr{g}{qc}", name=f"sr{g}{qc}"
                )

            def tail_norm_bcast(g, qc, piece):
                # broadcast one 128-col piece of the bounced sums over each
                # head's 32-row block. Pieces > 0 start one column early —
                # the overlap makes bcast[c] WAR-depend on recip[c-1]'s read,
                # dependency-chaining the pieces so the scheduler cannot
                # clump the reciprocals ahead of loop accumulates.
                sr = _srs[(g, qc)]
                pc = slice(256 * piece, 256 * (piece + 1))
                dc = slice(qc * 512 + 256 * piece, qc * 512 + 256 * (piece + 1))
                for j in range(4):
                    h = 4 * g + j
                    nc.sync.dma_start(
                        out=sr[32 * j : 32 * j + 32, pc],
                        in_=sums_d[h : h + 1, dc].broadcast_to((32, 256)),
                    )

            _rcs = {}

            def tail_norm_recip(g, qc, piece):
                # 1/rowsum, one 128-col piece (~0.9us on DVE), dep-chained
                # behind its bcast so pieces spread over the loop
                sr = _srs[(g, qc)]
                if (g, qc) not in _rcs:
                    _rcs[(g, qc)] = tailp.tile(
                        [DM, 512], f32, tag=f"rc{g}{qc}", name=f"rc{g}{qc}"
                    )
                pc = slice(256 * piece, 256 * (piece + 1))
                nc.vector.reciprocal(out=_rcs[(g, qc)][:, pc], in_=fv(sr[:, pc]))

            def tail_norm_mult(g, qc):
                # on GPSIMD: all-SBUF op, keeps the busy DVE queue clear
                sl = slice(qc * 512, (qc + 1) * 512)
                rc = _rcs.pop((g, qc))
                _srs.pop((g, qc))
                nc.gpsimd.tensor_tensor(
                    out=xs[g][:, sl],
                    in0=fv(xs[g][:, sl]),
                    in1=rc[:, :],
                    op=OP.mult,
                )

            with tc.tile_pool(name="sc_psum", bufs=2, space="PSUM") as sc_psum:

                def emit_sc(t):
                    g, kc, qc = ITERS[t]
                    st = sc_psum.tile([128, 4 * 512], f32, tag="sc", name="sc")
                    sct[t] = st
                    # j=0 last: its columns (0:512) hold the av output of the
                    # tile's previous occupant and are only freed once that
                    # accumulate has read them; j=1..3 only wait on the exp
                    for j in (1, 2, 3, 0):
                        nc.tensor.matmul(
                            out=st[:, 512 * j : 512 * (j + 1)],
                            lhsT=kcat[g][
                                32 * j : 32 * j + 32, kc * 128 : (kc + 1) * 128
                            ],
                            rhs=qcat[g][
                                32 * j : 32 * j + 32, qc * 512 : (qc + 1) * 512
                            ],
                            start=True,
                            stop=True,
                            tile_position=(32 * j, 0),
                        )

                emit_sc(0)
                emit_sc(1)
                for t in range(T):
                    g, kc, qc = ITERS[t]
                    e = exps.tile([128, 4 * 512], avdt, tag="e", name="e")
                    nc.scalar.activation(out=e[:, :], in_=sct[t][:, :], func=AF.Exp)
                    # attn @ [v|1|0..]: 4 col-tiled M=32 matmuls, reusing the
                    # consumed scores bank as output
                    for j in range(4):
                        h = 4 * g + j
                        nc.tensor.matmul(
                            out=sct[t][32 * j : 32 * j + 32, 0:512],
                            lhsT=v_aug[:, kc, 32 * h : 32 * h + 32],
                            rhs=e[:, 512 * j : 512 * (j + 1)],
                            start=True,
                            stop=True,
                            tile_position=(0, 32 * j),
                        )
                    # accumulate into xs (padding rows add exact zeros)
                    dstv = xs[g][:, qc * 512 : (qc + 1) * 512]
                    if kc == 0:  # first key chunk initializes xs (no memset)
                        nc.vector.tensor_copy(out=dstv, in_=sct[t][:, 0:512])
                    else:
                        nc.vector.tensor_tensor(
                            out=dstv, in0=fv(dstv), in1=sct[t][:, 0:512], op=OP.add
                        )
                    if t + 2 < T:
                        emit_sc(t + 2)
                    del sct[t]
                    if kc == NKC - 1:
                        tail_norm_scatter(g, qc)
                    if (g, qc) != (0, 0):
                        prev = (g, qc - 1) if qc else (g - 1, NQC - 1)
                        if kc in (1, 5):
                            tail_norm_bcast(*prev, (kc - 1) // 4)
                        elif kc in (3, 7):
                            tail_norm_recip(*prev, (kc - 3) // 4)
                        elif kc == 9:
                            tail_norm_mult(*prev)
                # last quarter's normalize (exposed; nothing left to hide it)
                for gq in list(_srs):
                    for piece in range(2):
                        tail_norm_bcast(*gq, piece)
                        tail_norm_recip(*gq, piece)
                    tail_norm_mult(*gq)

            # ---------------- output projection ----------------
            with tc.tile_pool(name="out_psum", bufs=1, space="PSUM") as out_psum:
                # chunked: matmul pair -> bias add -> store, per 512 columns
                ob = tailp.tile([DM, R], f32, tag="ob", name="ob")
                po = out_psum.tile([DM, R], f32, tag="po", name="po")
                for qc in range(NQC):
                    sl = slice(qc * 512, (qc + 1) * 512)
                    nc.tensor.matmul(
                        out=po[:, sl],
                        lhsT=s_woPA[:, :],
                        rhs=xs[0][:, sl],
                        start=True,
                        stop=False,
                    )
                    nc.tensor.matmul(
                        out=po[:, sl],
                        lhsT=s_woPB[:, :],
                        rhs=xs[1][:, sl],
                        start=False,
                        stop=True,
                    )
                    nc.vector.tensor_scalar_add(
                        out=ob[:, sl], in0=po[:, sl], scalar1=s_boc[:, :]
                    )
                    _ring[qc % 2].dma_start(out=outT_d[:, sl], in_=ob[:, sl])

    _split_multi_waits(nc, mybir)
    return nc


def _r12(a):
    """Round fp32 to float32r (11-bit mantissa, round-to-nearest-even) so the
    PE's f32r operand truncation is exact on DMA-fed tensors."""
    b = np.ascontiguousarray(a, np.float32).view(np.uint32)
    lsb = (b >> np.uint32(12)) & np.uint32(1)
    return ((b + np.uint32(0x7FF) + lsb) & np.uint32(0xFFFFF000)).view(np.float32)


def shard_inputs(query, key, value, pos_embed, W0, b0, W1, b1, W2, b2, Wo, bo):
    """Build the 8 per-core input maps (host-side layout preprocessing)."""
    f = np.float32
    asc = np.ascontiguousarray
    scale = 1.0 / np.sqrt(np.float32(DK))

    woPA = np.zeros((DM, DM), f)
    woPB = np.zeros((DM, DM), f)
    WoT = asc(Wo.T.astype(f))
    for j in range(4):
        woPA[32 * j + 1 : 32 * j + 17, :] = WoT[16 * j : 16 * j + 16, :]
        woPB[32 * j + 1 : 32 * j + 17, :] = WoT[16 * (4 + j) : 16 * (4 + j) + 16, :]

    shared = {
        "w0Ts": _r12(W0.T * scale),
        "w0T": _r12(W0.T),
        "w1T": _r12(W1.T),
        "w2T": _r12(W2.T),
        "woPA": _r12(woPA),
        "woPB": _r12(woPB),
        "bpack": asc(
            np.stack(
                [b0 * scale, b0, b1, bo], axis=1
            ).astype(f)
        ),
        "b2row": asc(b2.astype(f).reshape(1, DM)),
    }
    in_maps = []
    for c in range(NCORES):
        b_i, half = divmod(c, 2)
        r0 = half * R
        in_maps.append(
            dict(
                shared,
                xqT=_r12(query[b_i, r0 : r0 + R, :].T),
                posqT=_r12(pos_embed[b_i, r0 : r0 + R, :].T),
                xkT=_r12(key[b_i].T),
                xvT=_r12(value[b_i].T),
                posT=_r12(pos_embed[b_i].T),
            )
        )
    return in_maps


def gather_outputs(results):
    out = np.empty((B, S, DM), np.float32)
    for c in range(NCORES):
        b_i, half = divmod(c, 2)
        r0 = half * R
        out[b_i, r0 : r0 + R, :] = results[c]["outT"].T
    return out


def kernel(query, key, value, pos_embed, W0, b0, W1, b1, W2, b2, Wo, bo):
    from concourse.bass_utils import run_bass_kernel_spmd

    if "nc" not in _CACHE:
        _CACHE["nc"] = build_bass("float32r")
    in_maps = shard_inputs(
        query, key, value, pos_embed, W0, b0, W1, b1, W2, b2, Wo, bo
    )
    res = run_bass_kernel_spmd(_CACHE["nc"], in_maps, core_ids=list(range(NCORES)))
    return gather_outputs(res.results)



# revision 82
# speedup vs baseline: 1.0249x; 1.0249x over previous
"""Multi-head attention with additive positional attention — TRN2 Bass kernel.

Problem: B=4, S=2048, DM=128, H=8, DK=16.
  scores = (q @ k^T) / sqrt(DK) + pos_q @ pos_k^T   per (b, h)
  out    = softmax(scores) @ v, heads merged, @ Wo^T + bo

Sharding: 8 cores = batch (4) x query-row halves (2). Each core holds one
batch's full keys/values (S=2048) and 1024 query rows, computes all 8 heads,
and produces complete output rows — no cross-core reduction; the host gather
is a pure concatenation.

Per-core algorithm (all feature-major "T" layouts = [feature, seq]):
  - kcat/qcat: per head h, a 32-partition block [k_h (16 rows); pos_k_h (16)]
    (resp. [q_h * scale; pos_q_h]) so scoresT = kcat_blk^T @ qcat_blk fuses
    the qk and positional terms into ONE K=32 matmul per tile, 4 heads
    row-tiled concurrently on the PE's 32-row groups.
  - softmax without max-subtraction (scores are O(30), exp is fp32-safe).
  - v is augmented to 32 columns per head [1 | v_h | 0*15]: attn@v, the
    softmax row-sums, and hard zeros for the padding rows all come from one
    col-tiled matmul (M=32).
  - normalization (divide by row-sum) commutes with the output projection,
    so it's applied once at the end; Wo is host-permuted to read the
    scattered [head-block @ 32j] layout directly.
"""

import numpy as np

H, DK, DM = 8, 16, 128
B, S = 4, 2048
R = 1024  # query rows per core
NCORES = 8
NKC = S // 128  # 16 key chunks
NQC = R // 512  # 2 q chunks

_CACHE = {}


def _patch_drain():
    """walrus on this stack rejects >1 sync-wait on CTRL instructions; the
    TileContext exit drain can carry several. Absorb them on SP nops first."""
    import concourse.mybir as mybir
    from concourse.tile import TileContext, ScopedClock

    if getattr(TileContext, "_drain_patched", False):
        return
    orig = TileContext._drain_and_barrier

    def patched(self, tick_clock, wait_clock):
        nc = self.nc
        probe = nc.sync.nop(nofuse=True)
        wait_clock.add_sem_waits(
            probe.ins, ScopedClock({None: tick_clock.global_clock})
        )
        w = list(probe.ins.sync_info.on_wait or []) if probe.ins.sync_info else []
        if len(w) > 1:
            probe.ins.sync_info.on_wait = w[:1]
            for i in range(1, len(w)):
                n2 = nc.sync.nop(nofuse=True)
                n2.ins.sync_info = mybir.SyncInfo(on_wait=w[i : i + 1], on_update=[])

        class _NoWaits:
            def __init__(s, real):
                s._real = real

            def add_sem_waits(s, ins, clock):
                pass

            def __getattr__(s, k):
                return getattr(s._real, k)

        orig(self, tick_clock, _NoWaits(wait_clock))

    TileContext._drain_and_barrier = patched
    TileContext._drain_patched = True


def _split_multi_waits(nc, mybir):
    """walrus here accepts at most 1 sync-wait on most instruction structs
    (2 on EventSemaphore). Hoist excess waits onto same-engine NoOps placed
    immediately before the instruction — same blocking semantics."""
    for f in nc.m.functions:
        for blk in f.blocks:
            new_insts = []
            changed = False
            for inst in blk.instructions:
                si = inst.sync_info
                waits = list(si.on_wait) if si and si.on_wait else []
                limit = 2 if type(inst).__name__ == "InstEventSemaphore" else 1
                if len(waits) > limit:
                    changed = True
                    extra = waits[: len(waits) - limit]
                    for wv in extra:
                        n = mybir.InstNoOp(
                            name=f"wsplit_{nc.next_id()}",
                            engine=inst.engine,
                            ins=[],
                            outs=[],
                            sync_info=mybir.SyncInfo(on_wait=[wv], on_update=[]),
                        )
                        nc.register_instruction(n)
                        new_insts.append(n)
                    inst.sync_info.on_wait = waits[len(waits) - limit :]
                new_insts.append(inst)
            if changed:
                blk.instructions = new_insts


def build_bass(mm_dtype="float32"):
    import concourse.bass as bass
    import concourse.mybir as mybir
    import concourse.tile as tile

    _patch_drain()
    dt = mybir.dt
    f32 = dt.float32
    # mdt: dtype of every tensor that feeds a matmul. float32r = fp32 with
    # 11-bit mantissa (PE fast path, 1 cyc/col vs 4 for fp32 at N>=256).
    # The BIR verifier requires each matmul operand's PRODUCER to declare
    # float32r output (engines round on write; DMA chains end at an f32r
    # ExternalInput that the host pre-rounds).
    mdt = getattr(dt, mm_dtype)
    # f32r matmuls require dst partition offset 0 — the 4 col-tiled av
    # matmuls write bands at partitions 32j, so they use bf16 operands
    # instead (same 1 cyc/col rate, no dst restriction; attn weights and v
    # tolerate bf16 rounding easily at rel_fro < 2e-2).
    avdt = dt.bfloat16 if mdt != f32 else f32
    AF = mybir.ActivationFunctionType
    OP = mybir.AluOpType

    def fv(ap):  # f32 view of a (possibly f32r) AP for non-matmul readers
        return ap if mdt == f32 else ap.bitcast(f32)

    nc = bass.Bass("TRN2", num_devices=NCORES, enable_asserts=False)

    def inp(name, shape, dtype=f32):
        return nc.dram_tensor(name, shape, dtype, kind="ExternalInput")

    xqT_d = inp("xqT", [DM, R], mdt)
    posqT_d = inp("posqT", [DM, R], mdt)
    xkT_d = inp("xkT", [DM, S], mdt)
    xvT_d = inp("xvT", [DM, S], mdt)
    posT_d = inp("posT", [DM, S], mdt)
    w0Ts_d = inp("w0Ts", [DM, DM], mdt)
    w0T_d = inp("w0T", [DM, DM], mdt)
    w1T_d = inp("w1T", [DM, DM], mdt)
    w2T_d = inp("w2T", [DM, DM], mdt)
    woPA_d = inp("woPA", [DM, DM], mdt)
    woPB_d = inp("woPB", [DM, DM], mdt)
    # biases packed into one [DM, 4] DMA (cols: b0s, b0c, b1c, bo) and b2 as
    # a single-partition row (replicated on-device) — [DM,1] loads cost 128
    # tiny descriptors each and serialized ~2.5us apiece on the SP ring
    bpack_d = inp("bpack", [DM, 4])
    b2row_d = inp("b2row", [1, DM])
    outT_d = nc.dram_tensor("outT", [DM, R], f32, kind="ExternalOutput")
    sums_d = nc.dram_tensor("sums_scratch", [H, R], mdt)

    with tile.TileContext(nc) as tc:
        with (
            tc.tile_pool(name="singles", bufs=1) as singles,
            tc.tile_pool(name="exps", bufs=4) as exps,
            tc.tile_pool(name="tailp", bufs=1) as tailp,
        ):
            # ---------------- input loads ----------------
            # loads: SP ring (need-ordered); ACT ring carries only the
            # phase-1 interleave pieces (its in-order queue must never make
            # a load wait behind a piece that needs projection results)
            _ring = [nc.sync, nc.scalar]

            def load(dram, shape, ring=None):
                t = singles.tile(
                    shape, dram.dtype, tag=dram.name + "_s", name=dram.name + "_s"
                )
                (ring or nc.sync).dma_start(out=t[:, :], in_=dram[:, :])
                return t

            s_w1T = load(w1T_d, [DM, DM])
            s_xkT = load(xkT_d, [DM, S], nc.scalar)
            s_w0Ts = load(w0Ts_d, [DM, DM])
            s_xqT = load(xqT_d, [DM, R], nc.scalar)
            s_bpack = load(bpack_d, [DM, 4])
            s_posT = load(posT_d, [DM, S], nc.scalar)
            s_w0T = load(w0T_d, [DM, DM])
            s_posqT = load(posqT_d, [DM, R], nc.scalar)
            s_w2T = load(w2T_d, [DM, DM])
            s_xvT = load(xvT_d, [DM, S], nc.scalar)
            s_woPA = load(woPA_d, [DM, DM])
            s_woPB = load(woPB_d, [DM, DM], nc.scalar)
            s_b0s = s_bpack[:, 0:1]
            s_b0c = s_bpack[:, 1:2]
            s_b1c = s_bpack[:, 2:3]
            s_boc = s_bpack[:, 3:4]
            s_b2r = singles.tile([DM, DM], f32, tag="b2r_s", name="b2r_s")
            nc.scalar.dma_start(
                out=s_b2r[:, :], in_=b2row_d[0:1, :].broadcast_to((DM, DM))
            )

            kcat = [
                singles.tile([DM, S], mdt, tag="kcatA", name="kcatA"),
                singles.tile([DM, S], mdt, tag="kcatB", name="kcatB"),
            ]
            qcat = [
                singles.tile([DM, R], mdt, tag="qcatA", name="qcatA"),
                singles.tile([DM, R], mdt, tag="qcatB", name="qcatB"),
            ]
            # per head: [v_h (16) | 1 | 0*15] -> av matmul M=32 writes hard
            # zeros into the padding rows of each 32-block
            v_aug = singles.tile([DM, NKC, 32 * H], avdt, tag="vaug", name="vaug")
            xs = [
                singles.tile([DM, R], mdt, tag="xsA", name="xsA"),
                singles.tile([DM, R], mdt, tag="xsB", name="xsB"),
            ]
            nc.gpsimd.memset(v_aug[:, :, :], 0.0)
            nc.gpsimd.memset(
                v_aug.rearrange("p t (h u) -> p t h u", u=32)[:, :, :, 0], 1.0
            )

            # ---------------- projections ----------------
            # full feature-major projections into SBUF scratch, then DMA
            # partition-interleave into the per-head-block kcat/qcat layout
            kT_sb = singles.tile([DM, S], mdt, tag="kT_sb", name="kT_sb")
            pkT_sb = singles.tile([DM, S], mdt, tag="pkT_sb", name="pkT_sb")
            qT_sb = singles.tile([DM, R], mdt, tag="qT_sb", name="qT_sb")
            pqT_sb = singles.tile([DM, R], mdt, tag="pqT_sb", name="pqT_sb")

            with tc.tile_pool(name="proj_psum", bufs=4, space="PSUM") as proj_psum:
                def proj_chunk(lhsT, rhs_src, c0, bias, dst_sb):
                    pk = proj_psum.tile([128, 512], f32, tag="proj", name="pk")
                    nc.tensor.matmul(
                        out=pk[:, :],
                        lhsT=lhsT[:, :],
                        rhs=rhs_src[:, c0 : c0 + 512],
                        start=True,
                        stop=True,
                    )
                    # evac + bias in one op
                    nc.vector.tensor_scalar_add(
                        out=dst_sb[:, c0 : c0 + 512],
                        in0=pk[:, :],
                        scalar1=bias[:, :],
                    )

                # interleave pieces: kcat[g][32j + 16half + d] =
                # (k if half==0 else pos_k)[16(4g+j)+d], per (j, half),
                # column-phased so early-needed ranges land first
                def kpieces(g, cols, rings):
                    for j in range(4):
                        h = 4 * g + j
                        for half, src in ((0, kT_sb), (1, pkT_sb)):
                            r0 = 32 * j + 16 * half
                            rings[(j + half) % len(rings)].dma_start(
                                out=kcat[g][r0 : r0 + 16, cols],
                                in_=src[16 * h : 16 * h + 16, cols],
                            )

                def qpieces(g, cols, rings):
                    for j in range(4):
                        h = 4 * g + j
                        for half, src in ((0, qT_sb), (1, pqT_sb)):
                            r0 = 32 * j + 16 * half
                            rings[(j + half) % len(rings)].dma_start(
                                out=qcat[g][r0 : r0 + 16, cols],
                                in_=src[16 * h : 16 * h + 16, cols],
                            )

                # round-robin chunk 0 of each projection, then phase-1 pieces
                both = [nc.sync, nc.scalar]
                sp = [nc.sync]
                proj_chunk(s_w1T, s_xkT, 0, s_b1c, kT_sb)
                proj_chunk(s_w1T, s_posT, 0, s_b1c, pkT_sb)
                proj_chunk(s_w0Ts, s_xqT, 0, s_b0s, qT_sb)
                proj_chunk(s_w0T, s_posqT, 0, s_b0c, pqT_sb)
                kpieces(0, slice(0, 512), both)
                qpieces(0, slice(0, 512), both)
                proj_chunk(s_w1T, s_xkT, 512, s_b1c, kT_sb)
                proj_chunk(s_w1T, s_posT, 512, s_b1c, pkT_sb)
                proj_chunk(s_w0Ts, s_xqT, 512, s_b0s, qT_sb)
                proj_chunk(s_w0T, s_posqT, 512, s_b0c, pqT_sb)
                qpieces(0, slice(512, R), both)
                for c0 in (1024, 1536):
                    proj_chunk(s_w1T, s_xkT, c0, s_b1c, kT_sb)
                    proj_chunk(s_w1T, s_posT, c0, s_b1c, pkT_sb)
                kpieces(0, slice(512, S), sp)

                # v projection for the first two key chunks only — the rest
                # is pipelined into the attention loop (PE has slack there)
                def vproj(t, psum_tile, pcols):
                    nc.tensor.matmul(
                        out=psum_tile[:, pcols],
                        lhsT=s_xvT[:, t * 128 : (t + 1) * 128],
                        rhs=s_w2T[:, :],
                        start=True,
                        stop=True,
                    )
                    nc.vector.tensor_tensor(
                        out=v_aug.rearrange("p t (h u) -> p t h u", u=32)[
                            :, t, :, 1:17
                        ],
                        in0=psum_tile[:, pcols].rearrange(
                            "p (h u) -> p h u", u=16
                        ),
                        in1=s_b2r.rearrange("p (h u) -> p h u", u=16),
                        op=OP.add,
                    )

                for t in range(NKC):
                    pv = proj_psum.tile([128, 512], f32, tag="proj", name="pv")
                    vproj(t, pv, slice(0, DM))

                # phase-2 interleave for g=1: SP ring, hides under the loop
                kpieces(1, slice(0, S), sp)
                qpieces(1, slice(0, R), sp)

            # ---------------- attention main loop ----------------
            # qc-outer: each (g,qc) output quarter finalizes at 1/4-points of
            # the loop, so its normalize chain hides under later iterations
            ITERS = [
                (g, kc, qc) for g in (0, 1) for qc in range(NQC) for kc in range(NKC)
            ]
            T = len(ITERS)
            sct = {}

            _srs = {}

            def tail_norm_scatter(g, qc, rings=(nc.sync,)):
                # head row-sums (rows 32j of xs) -> DRAM bounce rows. SP
                # ring during the loop (ACT ring would stall exps).
                sl = slice(qc * 512, (qc + 1) * 512)
                for j in range(4):
                    h = 4 * g + j
                    rings[j % len(rings)].dma_start(
                        out=sums_d[h : h + 1, sl],
                        in_=xs[g][32 * j : 32 * j + 1, sl],
                    )
                _srs[(g, qc)] = tailp.tile(
                    [DM, 512], mdt, tag=f"sr{g}{qc}", name=f"sr{g}{qc}"
                )

            def tail_norm_bcast(g, qc, piece, rings=(nc.sync,)):
                # broadcast one 128-col piece of the bounced sums over each
                # head's 32-row block. Pieces > 0 start one column early —
                # the overlap makes bcast[c] WAR-depend on recip[c-1]'s read,
                # dependency-chaining the pieces so the scheduler cannot
                # clump the reciprocals ahead of loop accumulates.
                sr = _srs[(g, qc)]
                pc = slice(256 * piece, 256 * (piece + 1))
                dc = slice(qc * 512 + 256 * piece, qc * 512 + 256 * (piece + 1))
                for j in range(4):
                    h = 4 * g + j
                    rings[j % len(rings)].dma_start(
                        out=sr[32 * j : 32 * j + 32, pc],
                        in_=sums_d[h : h + 1, dc].broadcast_to((32, 256)),
                    )

            _rcs = {}

            def tail_norm_recip(g, qc, piece):
                # 1/rowsum, one 128-col piece (~0.9us on DVE), dep-chained
                # behind its bcast so pieces spread over the loop
                sr = _srs[(g, qc)]
                if (g, qc) not in _rcs:
                    _rcs[(g, qc)] = tailp.tile(
                        [DM, 512], f32, tag=f"rc{g}{qc}", name=f"rc{g}{qc}"
                    )
                pc = slice(256 * piece, 256 * (piece + 1))
                nc.vector.reciprocal(out=_rcs[(g, qc)][:, pc], in_=fv(sr[:, pc]))

            def tail_norm_mult(g, qc, eng=None):
                # GPSIMD mid-loop (all-SBUF op, keeps the DVE queue clear);
                # DVE for the exposed last quarter (it is idle then, faster)
                sl = slice(qc * 512, (qc + 1) * 512)
                rc = _rcs.pop((g, qc))
                _srs.pop((g, qc))
                (eng or nc.gpsimd).tensor_tensor(
                    out=xs[g][:, sl],
                    in0=fv(xs[g][:, sl]),
                    in1=rc[:, :],
                    op=OP.mult,
                )

            with tc.tile_pool(name="sc_psum", bufs=2, space="PSUM") as sc_psum:

                def emit_sc(t):
                    g, kc, qc = ITERS[t]
                    st = sc_psum.tile([128, 4 * 512], f32, tag="sc", name="sc")
                    sct[t] = st
                    # j=0 last: its columns (0:512) hold the av output of the
                    # tile's previous occupant and are only freed once that
                    # accumulate has read them; j=1..3 only wait on the exp
                    for j in (1, 2, 3, 0):
                        nc.tensor.matmul(
                            out=st[:, 512 * j : 512 * (j + 1)],
                            lhsT=kcat[g][
                                32 * j : 32 * j + 32, kc * 128 : (kc + 1) * 128
                            ],
                            rhs=qcat[g][
                                32 * j : 32 * j + 32, qc * 512 : (qc + 1) * 512
                            ],
                            start=True,
                            stop=True,
                            tile_position=(32 * j, 0),
                        )

                emit_sc(0)
                emit_sc(1)
                for t in range(T):
                    g, kc, qc = ITERS[t]
                    e = exps.tile([128, 4 * 512], avdt, tag="e", name="e")
                    nc.scalar.activation(out=e[:, :], in_=sct[t][:, :], func=AF.Exp)
                    # attn @ [v|1|0..]: 4 col-tiled M=32 matmuls, reusing the
                    # consumed scores bank as output
                    for j in range(4):
                        h = 4 * g + j
                        nc.tensor.matmul(
                            out=sct[t][32 * j : 32 * j + 32, 0:512],
                            lhsT=v_aug[:, kc, 32 * h : 32 * h + 32],
                            rhs=e[:, 512 * j : 512 * (j + 1)],
                            start=True,
                            stop=True,
                            tile_position=(0, 32 * j),
                        )
                    # accumulate into xs (padding rows add exact zeros)
                    dstv = xs[g][:, qc * 512 : (qc + 1) * 512]
                    if kc == 0:  # first key chunk initializes xs (no memset)
                        nc.vector.tensor_copy(out=dstv, in_=sct[t][:, 0:512])
                    else:
                        nc.vector.tensor_tensor(
                            out=dstv, in0=fv(dstv), in1=sct[t][:, 0:512], op=OP.add
                        )
                    if t + 2 < T:
                        emit_sc(t + 2)
                    del sct[t]
                    if kc == NKC - 1:
                        if (g, qc) == (1, NQC - 1):  # after the last exp
                            tail_norm_scatter(g, qc, rings=(nc.sync, nc.scalar))
                        else:
                            tail_norm_scatter(g, qc)
                    if (g, qc) != (0, 0):
                        prev = (g, qc - 1) if qc else (g - 1, NQC - 1)
                        if kc in (1, 5):
                            tail_norm_bcast(*prev, (kc - 1) // 4)
                        elif kc in (3, 7):
                            tail_norm_recip(*prev, (kc - 3) // 4)
                        elif kc == 9:
                            tail_norm_mult(*prev)
                # last quarter's normalize (exposed — both DMA rings are
                # free now, and DVE is idle for the multiply)
                for gq in list(_srs):
                    for piece in range(2):
                        tail_norm_bcast(*gq, piece, rings=(nc.sync, nc.scalar))
                        tail_norm_recip(*gq, piece)
                    tail_norm_mult(*gq, eng=nc.vector)

            # ---------------- output projection ----------------
            with tc.tile_pool(name="out_psum", bufs=1, space="PSUM") as out_psum:
                # chunked: matmul pair -> bias add -> store, per 512 columns
                ob = tailp.tile([DM, R], f32, tag="ob", name="ob")
                po = out_psum.tile([DM, R], f32, tag="po", name="po")
                for qc in range(NQC):
                    sl = slice(qc * 512, (qc + 1) * 512)
                    nc.tensor.matmul(
                        out=po[:, sl],
                        lhsT=s_woPA[:, :],
                        rhs=xs[0][:, sl],
                        start=True,
                        stop=False,
                    )
                    nc.tensor.matmul(
                        out=po[:, sl],
                        lhsT=s_woPB[:, :],
                        rhs=xs[1][:, sl],
                        start=False,
                        stop=True,
                    )
                    nc.vector.tensor_scalar_add(
                        out=ob[:, sl], in0=po[:, sl], scalar1=s_boc[:, :]
                    )
                    _ring[qc % 2].dma_start(out=outT_d[:, sl], in_=ob[:, sl])

    _split_multi_waits(nc, mybir)
    return nc


def _r12(a):
    """Round fp32 to float32r (11-bit mantissa, round-to-nearest-even) so the
    PE's f32r operand truncation is exact on DMA-fed tensors."""
    b = np.ascontiguousarray(a, np.float32).view(np.uint32)
    lsb = (b >> np.uint32(12)) & np.uint32(1)
    return ((b + np.uint32(0x7FF) + lsb) & np.uint32(0xFFFFF000)).view(np.float32)


def shard_inputs(query, key, value, pos_embed, W0, b0, W1, b1, W2, b2, Wo, bo):
    """Build the 8 per-core input maps (host-side layout preprocessing)."""
    f = np.float32
    asc = np.ascontiguousarray
    scale = 1.0 / np.sqrt(np.float32(DK))

    woPA = np.zeros((DM, DM), f)
    woPB = np.zeros((DM, DM), f)
    WoT = asc(Wo.T.astype(f))
    for j in range(4):
        woPA[32 * j + 1 : 32 * j + 17, :] = WoT[16 * j : 16 * j + 16, :]
        woPB[32 * j + 1 : 32 * j + 17, :] = WoT[16 * (4 + j) : 16 * (4 + j) + 16, :]

    shared = {
        "w0Ts": _r12(W0.T * scale),
        "w0T": _r12(W0.T),
        "w1T": _r12(W1.T),
        "w2T": _r12(W2.T),
        "woPA": _r12(woPA),
        "woPB": _r12(woPB),
        "bpack": asc(
            np.stack(
                [b0 * scale, b0, b1, bo], axis=1
            ).astype(f)
        ),
        "b2row": asc(b2.astype(f).reshape(1, DM)),
    }
    in_maps = []
    for c in range(NCORES):
        b_i, half = divmod(c, 2)
        r0 = half * R
        in_maps.append(
            dict(
                shared,
                xqT=_r12(query[b_i, r0 : r0 + R, :].T),
                posqT=_r12(pos_embed[b_i, r0 : r0 + R, :].T),
                xkT=_r12(key[b_i].T),
                xvT=_r12(value[b_i].T),
                posT=_r12(pos_embed[b_i].T),
            )
        )
    return in_maps


def gather_outputs(results):
    out = np.empty((B, S, DM), np.float32)
    for c in range(NCORES):
        b_i, half = divmod(c, 2)
        r0 = half * R
        out[b_i, r0 : r0 + R, :] = results[c]["outT"].T
    return out


def kernel(query, key, value, pos_embed, W0, b0, W1, b1, W2, b2, Wo, bo):
    from concourse.bass_utils import run_bass_kernel_spmd

    if "nc" not in _CACHE:
        _CACHE["nc"] = build_bass("float32r")
    in_maps = shard_inputs(
        query, key, value, pos_embed, W0, b0, W1, b1, W2, b2, Wo, bo
    )
    res = run_bass_kernel_spmd(_CACHE["nc"], in_maps, core_ids=list(range(NCORES)))
    return gather_outputs(res.results)

